# revision 1
# baseline (speedup 1.0000x reference)
"""AttentionHead kernel for Trainium2 (Bass/Tile), SPMD over 8 NeuronCores.

Problem: single attention head, B=8, T=4096, C=1024, D=64, fp32 I/O.
Sharding: data-parallel over batch; core b computes batch element b.

Per-core pipeline:
  1. X [T,C] fp32 -> SWDGE cast-DMA -> bf16 SBUF tiles -> DMA-xbar transpose
     -> X^T [C,T] bf16 (PE contracts along partitions, so X^T is required).
  2. Projections on PE (bf16 in, fp32 PSUM): stationary [Wk^T|Wq^T] gives
     [K^T;Q^T] stacked, stationary [Wv^T|Wk^T] gives V^T. Q^T is shifted to
     partitions 0-63 with a small SBUF->SBUF DMA so QK matmuls line up.
     V^T is re-transposed to natural V [T,D] via DMA-xbar, with a ones
     column appended (folds the softmax denominator into the PV matmul).
  3. Attention in transposed tile layout: S^T[s-block, q-chunk] = K_b @ Q^T.
     exp on ScalarE with scale=1/sqrt(D) folded in; causal mask handled by
     column-restricting every diagonal tile plus one affine_select triangle;
     PV accumulates O^T = [V|1]^T @ E^T into PSUM (row 64 = denominator).
  4. O^T chunks: PE transpose back to [q,65], reciprocal + per-partition
     scalar multiply, DMA out.
"""

import os

import numpy as np

import concourse.bass as bass
import concourse.tile as tile
from concourse import bacc, mybir
from concourse.bass_utils import run_bass_kernel_spmd
from concourse.masks import make_identity, make_upper_triangular

B, T, C, D = 8, 4096, 1024, 64
NCORES = 8
PB = 128                 # partition block
NB = T // PB             # 32 t/s blocks
CB = C // PB             # 8 contraction blocks
QCH = 512                # q-chunk width
NQ = T // QCH            # 8 q-chunks
SCW = 1024               # superchunk width (t rows handled per pipeline step)
NSC = T // SCW           # 4 superchunks
BF16 = mybir.dt.bfloat16
F32 = mybir.dt.float32


def _build_attention(tc: tile.TileContext, out_ap, x_ap, wk_ap, wq_ap, wv_ap):
    nc = tc.nc
    import contextlib

    ctx = contextlib.ExitStack()
    with ctx:
        singles = ctx.enter_context(tc.tile_pool(name="singles", bufs=1))
        persist = ctx.enter_context(tc.tile_pool(name="persist", bufs=1))
        xnat = ctx.enter_context(tc.tile_pool(name="xnat", bufs=2))
        xtp = ctx.enter_context(tc.tile_pool(name="xtp", bufs=2))
        aux_bufs = int(os.environ.get("KERNEL_AUX_BUFS", "2"))
        s2_bufs = int(os.environ.get("KERNEL_S2_BUFS", "2"))
        o_bufs = int(os.environ.get("KERNEL_O_BUFS", "2"))
        pspool = ctx.enter_context(
            tc.tile_pool(name="pspool", bufs=aux_bufs, space="PSUM"))
        s2pool = ctx.enter_context(
            tc.tile_pool(name="s2pool", bufs=s2_bufs, space="PSUM"))
        opool = ctx.enter_context(
            tc.tile_pool(name="opool", bufs=o_bufs, space="PSUM"))
        epool = ctx.enter_context(tc.tile_pool(name="epool", bufs=6))
        osb = ctx.enter_context(tc.tile_pool(name="osb", bufs=3))
        small = ctx.enter_context(tc.tile_pool(name="small", bufs=8))

        # ---- weight prep ------------------------------------------------
        # cast-load the three [D, C] fp32 weights to bf16 (SWDGE casts).
        wk_bf = singles.tile([D, C], BF16, tag="wk_bf")
        wq_bf = singles.tile([D, C], BF16, tag="wq_bf")
        wv_bf = singles.tile([D, C], BF16, tag="wv_bf")
        nc.gpsimd.dma_start(out=wk_bf, in_=wk_ap)
        nc.gpsimd.dma_start(out=wq_bf, in_=wq_ap)
        nc.gpsimd.dma_start(out=wv_bf, in_=wv_ap)

        identity = singles.tile([PB, PB], F32, tag="identity")
        make_identity(nc, identity)
        identity_bf = singles.tile([PB, PB], BF16, tag="identity_bf")
        make_identity(nc, identity_bf)
        # 0/1 upper-triangular (incl diagonal) mask for the causal edge
        tri_bf = singles.tile([PB, PB], BF16, tag="tri_bf")
        make_upper_triangular(nc, tri_bf, val=1.0, diag=True)

        # Stationary A: [Wk^T | Wq^T] per contraction block -> rows 0-63 of
        # the proj output are K^T, rows 64-127 are Q^T.
        # Stationary Bv: [Wv^T | Wk^T] -> rows 0-63 are V^T.
        # Transposed on PE (DMA-xbar transposes only tolerate one sync wait,
        # which the multi-source weight staging here would exceed).
        wa = singles.tile([PB, CB, PB], BF16, tag="wa")
        wb = singles.tile([PB, CB, PB], BF16, tag="wb")
        for cb in range(CB):
            csl = slice(cb * PB, (cb + 1) * PB)
            for src, dst in ((wk_bf, wa[:, cb, 0:D]), (wq_bf, wa[:, cb, D:PB]),
                             (wv_bf, wb[:, cb, 0:D]), (wk_bf, wb[:, cb, D:PB])):
                wt_ps = pspool.tile([PB, D], BF16, tag="ps", name="wt_ps")
                nc.tensor.transpose(wt_ps, src[:, csl], identity_bf[0:D, 0:D])
                nc.vector.tensor_copy(dst, wt_ps)

        # ---- per-superchunk persistent projection outputs ---------------
        kq_sc = [persist.tile([PB, SCW], BF16, tag=f"kq{sc}", name=f"kq{sc}")
                 for sc in range(NSC)]
        q0_sc = [persist.tile([D, SCW], BF16, tag=f"q0{sc}", name=f"q0{sc}")
                 for sc in range(NSC)]
        vt_sc = [persist.tile([D, SCW], BF16, tag=f"vt{sc}", name=f"vt{sc}")
                 for sc in range(NSC)]
        # natural V with a ones column: [128, 8 blocks, 80] per superchunk
        # (stride 80*2B keeps every block slice 32B aligned for the xbar)
        vn_sc = [persist.tile([PB, SCW // PB, 80], BF16, tag=f"vn{sc}", name=f"vn{sc}")
                 for sc in range(NSC)]

        xt_mode = os.environ.get("KERNEL_XT_MODE", "pe")

        x_blocked = x_ap.rearrange("(blk p) c -> p blk c", p=PB)

        # ---- stage 1: load X, transpose, project ------------------------
        def emit_proj(sc):
            xt = xtp.tile([PB, CB, SCW], BF16, tag="xt", name="xt")
            # one cast-load per half superchunk (few large SWDGE loads:
            # Q7 descriptor generation for many small ones gated the
            # pipeline; halves let transposes start at 2MB not 4MB)
            xbig = xnat.tile([PB, SCW // PB, C], BF16, tag="xb", name="xb")
            nb2 = SCW // PB // 2
            for hf in range(2):
                nc.gpsimd.dma_start(
                    out=xbig[:, hf * nb2:(hf + 1) * nb2, :],
                    in_=x_blocked[:, sc * (SCW // PB) + hf * nb2:
                                  sc * (SCW // PB) + (hf + 1) * nb2, :])
            xbs = [xbig[:, tb, :] for tb in range(SCW // PB)]
            if xt_mode in ("xbar", "xbar2"):
                for tb in range(SCW // PB):
                    for cb in range(CB):
                        eng = nc.sync if (xt_mode == "xbar" or cb % 2 == 0) \
                            else nc.scalar
                        eng.dma_start(
                            out=xt[:, cb, tb * PB:(tb + 1) * PB],
                            in_=xbs[tb][:, cb * PB:(cb + 1) * PB],
                            transpose=True,
                        )
            if xt_mode in ("pe", "mix"):
                # transpose on PE: 4 t-blocks of one c-block collect in one
                # PSUM tile, drained with a single DVE copy; in "mix" mode
                # odd c-blocks go through the DMA xbar instead
                for half in range(SCW // QCH):
                    for cb in range(CB):
                        if xt_mode == "mix" and cb % 2 == 1:
                            for u in range(QCH // PB):
                                tb = half * (QCH // PB) + u
                                nc.sync.dma_start(
                                    out=xt[:, cb, tb * PB:(tb + 1) * PB],
                                    in_=xbs[tb][:, cb * PB:(cb + 1) * PB],
                                    transpose=True,
                                )
                            continue
                        t_ps = pspool.tile([PB, QCH], BF16, tag="ps",
                                           name="t_ps")
                        for u in range(QCH // PB):
                            tb = half * (QCH // PB) + u
                            nc.tensor.transpose(
                                t_ps[:, u * PB:(u + 1) * PB],
                                xbs[tb][:, cb * PB:(cb + 1) * PB],
                                identity_bf,
                            )
                        nc.vector.tensor_copy(
                            xt[:, cb, half * QCH:(half + 1) * QCH], t_ps)
            for nch in range(SCW // QCH):
                nsl = slice(nch * QCH, (nch + 1) * QCH)
                kq_ps = pspool.tile([PB, QCH], F32, tag="ps")
                for cb in range(CB):
                    nc.tensor.matmul(
                        kq_ps, lhsT=wa[:, cb, :], rhs=xt[:, cb, nsl],
                        start=(cb == 0), stop=(cb == CB - 1),
                    )
                nc.scalar.activation(
                    out=kq_sc[sc][:, nsl], in_=kq_ps,
                    func=mybir.ActivationFunctionType.Copy,
                )
                v_ps = pspool.tile([PB, QCH], F32, tag="ps")
                for cb in range(CB):
                    nc.tensor.matmul(
                        v_ps, lhsT=wb[:, cb, :], rhs=xt[:, cb, nsl],
                        start=(cb == 0), stop=(cb == CB - 1),
                    )
                nc.scalar.activation(
                    out=vt_sc[sc][:, nsl], in_=v_ps[0:D, :],
                    func=mybir.ActivationFunctionType.Copy)

                # Q^T shifted to partitions 0-63 via PE identity matmul
                # (a plain SBUF->SBUF DMA would force xbar-mode transitions
                # against the transpose stream)
                q0_ps = pspool.tile([D, QCH], F32, tag="ps", name="q0_ps")
                nc.tensor.matmul(
                    q0_ps, lhsT=identity_bf[D:PB, D:PB],
                    rhs=kq_sc[sc][D:PB, nsl], start=True, stop=True,
                )
                nc.scalar.activation(
                    out=q0_sc[sc][:, nsl], in_=q0_ps,
                    func=mybir.ActivationFunctionType.Copy)

            # natural V blocks via xbar transpose
            for tb in range(SCW // PB):
                nc.sync.dma_start(
                    out=vn_sc[sc][:, tb, 0:D],
                    in_=vt_sc[sc][:, tb * PB:(tb + 1) * PB],
                    transpose=True,
                )
            # ones column for the folded softmax denominator (after the
            # transposes so they keep a single sync wait each)
            nc.gpsimd.memset(vn_sc[sc][:, :, D:D + 1], 1.0)

        # ---- stage 2: attention -----------------------------------------
        # s-blocks processed in pairs: two QK matmuls fill one [128, 1024]
        # PSUM tile, ONE exp covers both (the ACT sequencer's ~1.1us/op
        # issue+event-sem overhead is the critical path, so fewer, larger
        # activations win)
        out_blocked = out_ap.rearrange("(nb p) d -> nb p d", p=PB)
        dve_exp_phase = int(os.environ.get("KERNEL_DVE_EXP", "-1"))
        dve_exp_jmin = int(os.environ.get("KERNEL_DVE_EXP_JMIN", "4"))

        def emit_attn(j):
            o_ps = opool.tile([D + 1, QCH], F32, tag="ops")
            nblk = 4 * j + 4
            sc_j, nch_j = (j * QCH) // SCW, ((j * QCH) % SCW) // QCH
            for bp in range(nblk // 2):
                halves = []
                for idx, b in ((0, 2 * bp), (1, 2 * bp + 1)):
                    r = b - 4 * j
                    c0 = 128 * r if r > 0 else 0
                    halves.append((idx, b, c0))
                s2 = s2pool.tile([PB, 2 * QCH], F32, tag="s2")
                for idx, b, c0 in halves:
                    nc.tensor.matmul(
                        s2[:, idx * QCH + c0:(idx + 1) * QCH],
                        lhsT=kq_sc[b // (SCW // PB)][
                            0:D, (b % (SCW // PB)) * PB:
                                 (b % (SCW // PB) + 1) * PB],
                        rhs=q0_sc[sc_j][:, nch_j * QCH + c0:(nch_j + 1) * QCH],
                        start=True, stop=True, skip_group_check=True,
                    )
                e_sb = epool.tile([PB, 2 * QCH], BF16, tag="e")
                escale = 1.0 / float(np.sqrt(D))
                if bp >= 2 * j:
                    # diagonal pair: the two written column ranges have a
                    # gap of unwritten PSUM between them -> exp per half
                    for idx, b, c0 in halves:
                        nc.scalar.activation(
                            out=e_sb[:, idx * QCH + c0:(idx + 1) * QCH],
                            in_=s2[:, idx * QCH + c0:(idx + 1) * QCH],
                            func=mybir.ActivationFunctionType.Exp,
                            scale=escale,
                        )
                elif dve_exp_phase >= 0 and j >= dve_exp_jmin \
                        and bp % 4 == dve_exp_phase:
                    # DVE offload: exp(s*x) ~= 1 + sx(1 + sx/2), exact to
                    # ~5e-6 for |sx| <= 0.05 (logits here are tiny); the
                    # ScalarE activation pipe is the attention bottleneck,
                    # so a slice of tiles computes exp polynomially on DVE
                    u_sb = osb.tile([PB, 2 * QCH], F32, tag="u", name="u_sb")
                    # u = x*(s^2/2) + s ; v = x*u ; e = v + 1
                    nc.vector.tensor_scalar(
                        out=u_sb, in0=s2, scalar1=0.5 * escale * escale,
                        scalar2=escale, op0=mybir.AluOpType.mult,
                        op1=mybir.AluOpType.add)
                    v_sb = epool.tile([PB, 2 * QCH], BF16, tag="v",
                                      name="v_sb")
                    nc.vector.tensor_mul(v_sb, s2, u_sb)
                    nc.vector.tensor_scalar_add(e_sb, v_sb, 1.0)
                else:
                    nc.scalar.activation(
                        out=e_sb, in_=s2,
                        func=mybir.ActivationFunctionType.Exp,
                        scale=escale,
                    )
                for idx, b, c0 in halves:
                    if b - 4 * j >= 0:
                        # causal edge: zero strictly-below-diagonal entries
                        nc.vector.tensor_mul(
                            e_sb[:, idx * QCH + c0:idx * QCH + c0 + PB],
                            e_sb[:, idx * QCH + c0:idx * QCH + c0 + PB],
                            tri_bf)
                for idx, b, c0 in halves:
                    nc.tensor.matmul(
                        o_ps[:, c0:QCH],
                        lhsT=vn_sc[b // (SCW // PB)][
                            :, b % (SCW // PB), 0:D + 1],
                        rhs=e_sb[:, idx * QCH + c0:(idx + 1) * QCH],
                        start=(b == 0), stop=(b == nblk - 1),
                        skip_group_check=True,
                    )
            o_sb = osb.tile([D + 1, QCH], F32, tag="osb")
            nc.vector.tensor_copy(o_sb, o_ps)
            for u in range(QCH // PB):
                ot_ps = pspool.tile([PB, D + 1], F32, tag="ps", name="ot_ps")
                nc.tensor.transpose(
                    out=ot_ps, in_=o_sb[:, u * PB:(u + 1) * PB],
                    identity=identity[0:D + 1, 0:D + 1],
                )
                ot_sb = small.tile([PB, D + 1], F32, tag="otsb")
                nc.vector.tensor_copy(ot_sb, ot_ps)
                rden = small.tile([PB, 1], F32, tag="rden")
                nc.vector.reciprocal(rden, ot_sb[:, D:D + 1])
                of = small.tile([PB, D], F32, tag="of")
                nc.vector.tensor_scalar_mul(of, ot_sb[:, 0:D], rden)
                nc.sync.dma_start(out=out_blocked[4 * j + u], in_=of)

        # attention chunk j depends only on superchunks <= j//2: emit each
        # chunk right after its prerequisites so the scheduler overlaps the
        # ACT-heavy attention with the PE/DMA-heavy projection stream
        for sc in range(NSC):
            emit_proj(sc)
            if sc >= 1:
                emit_attn(2 * (sc - 1))
                emit_attn(2 * (sc - 1) + 1)
        for j in range(2 * (NSC - 1), NQ):
            emit_attn(j)


_NC_CACHE = {}


def _split_dma_transpose_waits(nc):
    """This walrus build accepts only ONE sync-wait command on DMA-queue
    instructions (DMA_DIRECT2D/XPOSE/CTRL_NO structs); Tile's sem
    assignment sometimes attaches 2-8. Move every wait from multi-wait
    DMA instructions onto same-queue InstNoOps inserted right before
    (same engine FIFO => ordering holds)."""
    n_split = 0
    for f in nc.m.functions:
        for blk in f.blocks:
            insts = blk.instructions
            i = 0
            while i < len(insts):
                inst = insts[i]
                if isinstance(inst, mybir.InstDmaTransposeAnt) or type(
                        inst).__name__.startswith("InstDMA"):
                    si = inst.sync_info
                    if si is not None and len(si.on_wait) > 1:
                        waits = list(si.on_wait)
                        si.on_wait = []
                        for w0 in range(len(waits)):
                            nop = mybir.InstNoOp(
                                name=f"xposewait-{inst.name}-{w0}", ins=[], outs=[])
                            nop.engine = inst.engine
                            nop.sync_info = mybir.SyncInfo(
                                on_wait=[waits[w0]], on_update=[])
                            insts.insert(i, nop)
                            i += 1
                        n_split += 1
                i += 1
    return n_split


def _build_nc(compile=True):
    key = ("nc", compile)
    if key in _NC_CACHE:
        return _NC_CACHE[key]
    nc = bacc.Bacc("TRN2", target_bir_lowering=False, debug=False)
    x_ap = nc.dram_tensor("X", [T, C], F32, kind="ExternalInput").ap()
    wk_ap = nc.dram_tensor("Wk", [D, C], F32, kind="ExternalInput").ap()
    wq_ap = nc.dram_tensor("Wq", [D, C], F32, kind="ExternalInput").ap()
    wv_ap = nc.dram_tensor("Wv", [D, C], F32, kind="ExternalInput").ap()
    out_ap = nc.dram_tensor("out", [T, D], F32, kind="ExternalOutput").ap()
    with tile.TileContext(nc) as tc:
        _build_attention(tc, out_ap, x_ap, wk_ap, wq_ap, wv_ap)
    if compile:
        nc.compile()
    _NC_CACHE[key] = nc
    return nc


def kernel(X, Wk, Wq, Wv):
    assert X.shape == (B, T, C), X.shape
    nc = _build_nc()
    X = np.ascontiguousarray(X, dtype=np.float32)
    in_maps = [
        {
            "X": X[b],
            "Wk": np.ascontiguousarray(Wk, dtype=np.float32),
            "Wq": np.ascontiguousarray(Wq, dtype=np.float32),
            "Wv": np.ascontiguousarray(Wv, dtype=np.float32),
        }
        for b in range(NCORES)
    ]
    res = run_bass_kernel_spmd(nc, in_maps, core_ids=list(range(NCORES)))
    if res.exec_time_ns is not None:
        print(f"[kernel] HW exec time: {res.exec_time_ns} ns "
              f"(mean {res.mean_exec_time_ns} ns)")
        if res.instructions_and_trace is not None:
            print(f"[kernel] trace: {res.instructions_and_trace[1]}")
    out = np.stack([res.results[b]["out"] for b in range(NCORES)], axis=0)
    return out

